# revision 5
# baseline (speedup 1.0000x reference)
"""Bass/Trainium2 kernel for nn_C4OpcodeExecutor (scatter_memory).

Contract: kernel(**inputs) takes the FULL unsharded inputs
(memory [2048, 65536] int64, read_addr/write_addr/value [2048] int64)
and returns the FULL output (read_val [2048] int64, new_mem [2048, 65536]
int64), matching reference.reference().

Strategy: pure data parallel over 8 NeuronCores; each core owns 256 rows.
Per core the kernel
  1. bulk-copies its 128 MiB memory shard DRAM->DRAM into the output,
  2. gathers 8 consecutive int64 "bytes" per row at read_addr via
     indirect DMA and packs their low bytes into the little-endian int64
     read_val,
  3. scatters 8 int64 elements per row (the zero-extended bytes of
     value) into the output at write_addr via indirect DMA, ordered
     after the bulk copy.
"""

import sys

import numpy as np

for _p in ("/opt/trn_rl_repo",):
    if _p not in sys.path:
        sys.path.insert(0, _p)

import jax

jax.config.update("jax_enable_x64", True)

from concourse import bacc, bass, mybir, tile
from concourse import bass_utils

B, M = 2048, 65536
NCORES = 8
BLOC = B // NCORES  # rows per core = 256
P = 128  # SBUF partitions
G = BLOC // P  # row groups per partition = 2


def build_nc():
    """Build the per-core Bass program (identical on all 8 cores)."""
    nc = bacc.Bacc("TRN2", target_bir_lowering=False, debug=False)

    i64 = mybir.dt.int64
    i32 = mybir.dt.int32
    u8 = mybir.dt.uint8

    mem_in = nc.dram_tensor("memory", [BLOC, M], i64, kind="ExternalInput")
    ra_in = nc.dram_tensor("read_addr32", [BLOC], i32, kind="ExternalInput")
    wa_in = nc.dram_tensor("write_addr32", [BLOC], i32, kind="ExternalInput")
    val_in = nc.dram_tensor("value", [BLOC], i64, kind="ExternalInput")
    mem_out = nc.dram_tensor("new_mem", [BLOC, M], i64, kind="ExternalOutput")
    rv_out = nc.dram_tensor("read_val", [BLOC], i64, kind="ExternalOutput")

    mem_flat = mem_in[:].rearrange("a b -> (a b)")[:, None]
    out_flat = mem_out[:].rearrange("a b -> (a b)")[:, None]
    rv_bytes = rv_out[:].bitcast(u8).rearrange("(r x) -> r x", x=8)
    val_bytes = val_in[:].bitcast(u8).rearrange("(r x) -> r x", x=8)

    with tile.TileContext(nc) as tc:
        with tc.tile_pool(name="sbuf", bufs=G) as pool:
            # Bulk copy memory -> new_mem (DRAM->DRAM, 128 MiB).
            nc.sync.dma_start(out=mem_out[:], in_=mem_in[:])

            # HW indirect DMA consumes ONE index per SBUF partition (the
            # first element of the offset AP's partition), moving that
            # partition's whole free-dim run contiguously. So: process
            # rows in G chunks of 128, one row per partition.
            for c in range(G):
                rows = slice(c * P, (c + 1) * P)

                # base[p] = (c*P + p) * M as int32 (iota steps are
                # int16-limited, so emit the row number and scale by M).
                rownum_t = pool.tile([P, 1], i32)
                nc.gpsimd.iota(
                    rownum_t[:], pattern=[[1, 1]], base=c * P, channel_multiplier=1
                )
                base_t = pool.tile([P, 1], i32)
                nc.vector.tensor_scalar_mul(base_t[:], rownum_t[:], M)

                ra_t = pool.tile([P, 1], i32)
                nc.sync.dma_start(out=ra_t[:], in_=ra_in[rows, None])
                wa_t = pool.tile([P, 1], i32)
                nc.sync.dma_start(out=wa_t[:], in_=wa_in[rows, None])

                ridx = pool.tile([P, 1], i32)
                nc.vector.tensor_add(ridx[:], base_t[:], ra_t[:])
                widx = pool.tile([P, 1], i32)
                nc.vector.tensor_add(widx[:], base_t[:], wa_t[:])

                # Gather 8 consecutive int64 elements per row from memory.
                gt = pool.tile([P, 8], i64)
                nc.gpsimd.indirect_dma_start(
                    out=gt[:],
                    out_offset=None,
                    in_=mem_flat,
                    in_offset=bass.IndirectOffsetOnAxis(ap=ridx[:], axis=0),
                )
                # Each gathered element holds a byte value; its low byte
                # (LE offset 0 of each int64) packed densely is read_val.
                rvb = pool.tile([P, 8], u8)
                nc.vector.tensor_copy(out=rvb[:], in_=gt[:].bitcast(u8)[:, 0:64:8])
                nc.sync.dma_start(out=rv_bytes[rows], in_=rvb[:])

                # Scatter payload: per row the 8 little-endian bytes of
                # value, each zero-extended to an int64 element.
                vb = pool.tile([P, 8], u8)
                nc.sync.dma_start(out=vb[:], in_=val_bytes[rows])
                sp = pool.tile([P, 8], i64)
                nc.gpsimd.memset(sp[:].bitcast(u8), 0)
                nc.vector.tensor_copy(out=sp[:].bitcast(u8)[:, 0:64:8], in_=vb[:])

                # Scatter 8 int64 per row into new_mem at write_addr.
                # Tile orders this after the bulk copy (WAW on mem_out).
                nc.gpsimd.indirect_dma_start(
                    out=out_flat,
                    out_offset=bass.IndirectOffsetOnAxis(ap=widx[:], axis=0),
                    in_=sp[:],
                    in_offset=None,
                )

    nc.compile()
    return nc


_NC_CACHE = []


def _get_nc():
    if not _NC_CACHE:
        _NC_CACHE.append(build_nc())
    return _NC_CACHE[0]


def make_in_maps(memory, read_addr, write_addr, value):
    """Shard the full inputs into per-core input maps."""
    in_maps = []
    for c in range(NCORES):
        sl = slice(c * BLOC, (c + 1) * BLOC)
        in_maps.append(
            {
                "memory": np.ascontiguousarray(memory[sl]),
                "read_addr32": read_addr[sl].astype(np.int32),
                "write_addr32": write_addr[sl].astype(np.int32),
                "value": np.ascontiguousarray(value[sl]),
            }
        )
    return in_maps


def kernel(memory, read_addr, write_addr, value, _trace=False, _trace_cores=None):
    memory = np.asarray(memory)
    read_addr = np.asarray(read_addr)
    write_addr = np.asarray(write_addr)
    value = np.asarray(value)
    assert memory.shape == (B, M) and memory.dtype == np.int64

    nc = _get_nc()
    in_maps = make_in_maps(memory, read_addr, write_addr, value)
    res = bass_utils.run_bass_kernel_spmd(
        nc,
        in_maps,
        core_ids=list(range(NCORES)),
        trace=_trace,
        trace_cores=_trace_cores,
    )
    read_val = np.concatenate([r["read_val"] for r in res.results])
    new_mem = np.concatenate([r["new_mem"] for r in res.results], axis=0)
    if _trace:
        kernel.last_results = res
    return read_val.astype(np.int64), new_mem.astype(np.int64)
